# revision 6
# baseline (speedup 1.0000x reference)
"""AttentionAggregator2d Trainium2 kernel.

Reference semantics (per batch sample; 8 samples, one per NeuronCore):
    zm [256, 4096];  q = Wq@zm+bq [32, 4096];  k likewise;  v = Wv@zm+bv [256, 4096]
    A = softmax_rows(q^T k)        # A[t, i] = exp(q_t.k_i) / sum_i' exp(q_t.k_i')
    out[c, i] = zc[c, i] + gamma * sum_t v[c, t] * A[t, i]

Device algorithm (per core, data-parallel over batch):
  stage 1: projections via float32r matmuls. q is written replicated x4 across
    partition quadrants (stationary side of the score matmuls, K=32 row-packed
    via tile_position); k likewise (moving side must live in the same quadrant).
    v^T [t, c] built tile-by-tile (lhsT = zm chunk); bv folded in through a
    K=1 ones-row matmul into the same PSUM accumulation.
  stage 2 (4 runs of 8 t-tiles):
    scores S[t, i] for one t-tile computed 4-way packed ([128, 2048] PSUM,
    4 i-chunks of 512); ScalarE exp -> P~ in bf16 SBUF (10-slot ring), with
    accum_out giving the softmax denominator D[t] for free.
    uT[t, c] = v^T[t, c] * (gamma / D[t])  (per-partition scalars, bf16)
    out accumulation: acc[c-tile, i-chunk] = sum_t uT[t, c]^T P~[t, i] in PSUM
    over the run's 8 t-tiles, spilled to SBUF fp32 with zc added on run 0.
  No softmax max-subtraction: |S| <= ~45 for this distribution, exact in fp32.
"""

import numpy as np

N = 4096          # tokens (64*64)
C = 256           # channels (CM == CC)
P = 32            # q/k projection channels
NG = 16           # stage-1 token chunks of 256
TT = 128          # t-tile size
NTT = N // TT     # 32 t-tiles
RUN = 8           # t-tiles per accumulation run
NRUN = NTT // RUN # 4 runs
IC = 512          # i-chunk (output token chunk)
NIC = N // IC     # 8
PBLK_SLOTS = 10   # P~ ring slots (bf16 [128, 4096] each)
B = 8             # batch == cores

_cache = {}


def _build_module():
    import concourse.bacc as bacc
    import concourse.tile as tile
    from concourse import mybir
    from contextlib import ExitStack

    f32 = mybir.dt.float32
    f32r = mybir.dt.float32r
    bf16 = mybir.dt.bfloat16
    AF = mybir.ActivationFunctionType
    OP = mybir.AluOpType

    nc = bacc.Bacc(trn_type="TRN2", debug=False)

    zm_d = nc.dram_tensor("zm", [C, N], f32, kind="ExternalInput").ap()
    zc_d = nc.dram_tensor("zc", [C, N], f32, kind="ExternalInput").ap()
    wq_d = nc.dram_tensor("wq4", [C, 128], f32, kind="ExternalInput").ap()
    wk_d = nc.dram_tensor("wk4", [C, 128], f32, kind="ExternalInput").ap()
    wv_d = nc.dram_tensor("wvt", [C, C], f32, kind="ExternalInput").ap()
    bq_d = nc.dram_tensor("bq4", [128, 1], f32, kind="ExternalInput").ap()
    bk_d = nc.dram_tensor("bk4", [128, 1], f32, kind="ExternalInput").ap()
    bv_d = nc.dram_tensor("bvr", [1, C], f32, kind="ExternalInput").ap()
    gam_d = nc.dram_tensor("gam", [128, 1], f32, kind="ExternalInput").ap()
    one_d = nc.dram_tensor("ones", [1, 128], f32, kind="ExternalInput").ap()
    out_d = nc.dram_tensor("out", [C, N], f32, kind="ExternalOutput").ap()

    with tile.TileContext(nc) as tc, ExitStack() as ctx:
        consts = ctx.enter_context(tc.tile_pool(name="consts", bufs=1))
        zm_pool = ctx.enter_context(tc.tile_pool(name="zmp", bufs=6))
        big = ctx.enter_context(tc.tile_pool(name="big", bufs=1))
        p_pool = ctx.enter_context(tc.tile_pool(name="pblk", bufs=PBLK_SLOTS))
        ut_pool = ctx.enter_context(tc.tile_pool(name="ut", bufs=PBLK_SLOTS))
        d_pool = ctx.enter_context(tc.tile_pool(name="dp", bufs=2))
        zc_pool = ctx.enter_context(tc.tile_pool(name="zcp", bufs=4))
        ps_s = ctx.enter_context(tc.tile_pool(name="ps_s", bufs=1, space="PSUM"))
        ps_acc = ctx.enter_context(tc.tile_pool(name="ps_acc", bufs=4, space="PSUM"))

        # ---- constants ----
        wq_sb = consts.tile([128, 256], f32r, name="wq_sb")
        wk_sb = consts.tile([128, 256], f32r, name="wk_sb")
        wv_sb = consts.tile([128, 512], f32r, name="wv_sb")
        bq_sb = consts.tile([128, 1], f32, name="bq_sb")
        bk_sb = consts.tile([128, 1], f32, name="bk_sb")
        bv_sb = consts.tile([1, C], f32r, name="bv_sb")
        one_sb = consts.tile([1, 128], f32r, name="one_sb")
        gam_sb = consts.tile([128, 1], f32, name="gam_sb")
        for h in range(2):
            nc.sync.dma_start(out=wq_sb[:, h * 128:(h + 1) * 128],
                              in_=wq_d[h * 128:(h + 1) * 128, :].bitcast(f32r))
            nc.sync.dma_start(out=wk_sb[:, h * 128:(h + 1) * 128],
                              in_=wk_d[h * 128:(h + 1) * 128, :].bitcast(f32r))
            nc.sync.dma_start(out=wv_sb[:, h * 256:(h + 1) * 256],
                              in_=wv_d[h * 128:(h + 1) * 128, :].bitcast(f32r))
        nc.sync.dma_start(out=bq_sb, in_=bq_d)
        nc.sync.dma_start(out=bk_sb, in_=bk_d)
        nc.sync.dma_start(out=bv_sb, in_=bv_d.bitcast(f32r))
        nc.sync.dma_start(out=gam_sb, in_=gam_d)
        nc.sync.dma_start(out=one_sb, in_=one_d.bitcast(f32r))

        # ---- persistent tiles ----
        q_rep = big.tile([128, N], f32r, name="q_rep")
        k_rep = big.tile([128, N], f32r, name="k_rep")
        vt = big.tile([128, NTT * C], bf16, name="vt")          # v^T per t-tile [t, c]
        acc_sb = big.tile([128, 2 * N], f32, name="acc_sb")     # out staging [c-tile, i]

        # ---- stage 1: stream zm chunks, project q/k/v ----
        for g in range(NG):
            sl = slice(g * 256, (g + 1) * 256)
            zm_t = zm_pool.tile([128, 512], f32r, name="zm_t", tag="zm")
            nc.sync.dma_start(out=zm_t[:, 0:256], in_=zm_d[0:128, sl].bitcast(f32r))
            nc.sync.dma_start(out=zm_t[:, 256:512], in_=zm_d[128:256, sl].bitcast(f32r))
            # k (ScalarE evac with bias)
            psk = ps_acc.tile([128, 512], f32, name="psk", tag="acc")
            nc.tensor.matmul(psk[:, 0:256], wk_sb[:, 0:128],
                             zm_t[:, 0:256], start=True, stop=False)
            nc.tensor.matmul(psk[:, 0:256], wk_sb[:, 128:256],
                             zm_t[:, 256:512], start=False, stop=True)
            nc.scalar.activation(k_rep[:, sl], psk[:, 0:256], AF.Identity, bias=bk_sb)
            # q (VectorE evac with bias)
            psq = ps_acc.tile([128, 512], f32, name="psq", tag="acc")
            nc.tensor.matmul(psq[:, 0:256], wq_sb[:, 0:128],
                             zm_t[:, 0:256], start=True, stop=False)
            nc.tensor.matmul(psq[:, 0:256], wq_sb[:, 128:256],
                             zm_t[:, 256:512], start=False, stop=True)
            nc.vector.tensor_scalar_add(q_rep[:, sl], psq[:, 0:256], bq_sb)
            # v^T for the chunk's two t-tiles (bv via K=1 ones-row matmul)
            for s in range(2):
                tt = 2 * g + s
                psv = ps_acc.tile([128, 512], f32, name="psv", tag="acc")
                nc.tensor.matmul(psv[:, 0:256], one_sb,
                                 bv_sb, start=True, stop=False)
                nc.tensor.matmul(psv[:, 0:256], zm_t[:, s * 128:(s + 1) * 128],
                                 wv_sb[:, 0:256], start=False, stop=False)
                nc.tensor.matmul(psv[:, 0:256], zm_t[:, 256 + s * 128:256 + (s + 1) * 128],
                                 wv_sb[:, 256:512], start=False, stop=True)
                nc.vector.tensor_copy(vt[:, tt * C:(tt + 1) * C], psv[:, 0:256])

        # ---- stage 2: scores + exp + accumulation, 4 runs of 8 t-tiles ----
        for run in range(NRUN):
            pts = []
            uts = []
            dcol = d_pool.tile([128, 2 * RUN], f32, name="dcol", tag="dcol")
            for tl in range(RUN):
                tt = run * RUN + tl
                pt = p_pool.tile([128, N], bf16, name="pt", tag="pt")
                pts.append(pt)
                for half in range(2):
                    s = ps_s.tile([128, 2048], f32, name="s_sc", tag="s")
                    for r in range(4):
                        ic = half * 4 + r
                        nc.tensor.matmul(
                            s[:, r * 512:(r + 1) * 512],
                            q_rep[32 * r:32 * (r + 1), tt * TT:(tt + 1) * TT],
                            k_rep[32 * r:32 * (r + 1), ic * IC:(ic + 1) * IC],
                            start=True, stop=True, tile_position=(32 * r, 0),
                        )
                    nc.scalar.activation(pt[:, half * 2048:(half + 1) * 2048], s,
                                         AF.Exp, accum_out=dcol[:, tl * 2 + half:tl * 2 + half + 1])
            # D = sum of the two half-sums; uT = v^T * (gamma / D)
            dsum = d_pool.tile([128, RUN], f32, name="dsum", tag="dsum")
            dview = dcol.rearrange("p (t h) -> p t h", h=2)
            nc.vector.tensor_tensor(dsum, dview[:, :, 0], dview[:, :, 1], op=OP.add)
            drec = d_pool.tile([128, RUN], f32, name="drec", tag="drec")
            nc.vector.reciprocal(drec, dsum)
            for tl in range(RUN):
                tt = run * RUN + tl
                ut = ut_pool.tile([128, C], bf16, name="ut", tag="ut")
                uts.append(ut)
                nc.vector.tensor_scalar(ut, vt[:, tt * C:(tt + 1) * C],
                                        drec[:, tl:tl + 1], gam_sb,
                                        op0=OP.mult, op1=OP.mult)
            # out[c, i] accumulation over the run's t-tiles
            for c in range(2):
                for ic in range(NIC):
                    a = ps_acc.tile([128, 512], f32, name="a_out", tag="acc")
                    for tl in range(RUN):
                        nc.tensor.matmul(a, uts[tl][:, c * 128:(c + 1) * 128],
                                         pts[tl][:, ic * IC:(ic + 1) * IC],
                                         start=(tl == 0), stop=(tl == RUN - 1))
                    dst = acc_sb[:, c * N + ic * IC: c * N + (ic + 1) * IC]
                    if run == 0:
                        zcs = zc_pool.tile([128, 512], f32, name="zcs", tag="zc")
                        nc.sync.dma_start(out=zcs, in_=zc_d[c * 128:(c + 1) * 128,
                                                            ic * IC:(ic + 1) * IC])
                        nc.vector.tensor_tensor(dst, a, zcs, op=OP.add)
                    else:
                        nc.vector.tensor_tensor(dst, a, dst, op=OP.add)
                    if run == NRUN - 1:
                        nc.sync.dma_start(out=out_d[c * 128:(c + 1) * 128,
                                                    ic * IC:(ic + 1) * IC], in_=dst)

    nc.compile()
    return nc


def _get_module():
    if "nc" not in _cache:
        _cache["nc"] = _build_module()
    return _cache["nc"]


def _host_prep(Wq, bq, Wk, bk, Wv, bv, gamma):
    g = np.float32(np.asarray(gamma).reshape(-1)[0])
    wq4 = np.ascontiguousarray(np.tile(np.asarray(Wq).T.astype(np.float32), (1, 4)))
    wk4 = np.ascontiguousarray(np.tile(np.asarray(Wk).T.astype(np.float32), (1, 4)))
    wvt = np.ascontiguousarray(np.asarray(Wv).T.astype(np.float32))
    bq4 = np.ascontiguousarray(np.tile(np.asarray(bq).astype(np.float32), 4).reshape(128, 1))
    bk4 = np.ascontiguousarray(np.tile(np.asarray(bk).astype(np.float32), 4).reshape(128, 1))
    bvr = np.ascontiguousarray(np.asarray(bv).astype(np.float32).reshape(1, C))
    gam = np.full((128, 1), g, np.float32)
    ones = np.ones((1, 128), np.float32)
    return dict(wq4=wq4, wk4=wk4, wvt=wvt, bq4=bq4, bk4=bk4, bvr=bvr, gam=gam, ones=ones)


def kernel(zc, zm, Wq, bq, Wk, bk, Wv, bv, gamma):
    from concourse.bass_utils import run_bass_kernel_spmd

    zc = np.asarray(zc)
    zm = np.asarray(zm)
    b, c, w, h = zm.shape
    assert (b, c, w * h) == (B, C, N), (zm.shape,)
    nc = _get_module()
    shared = _host_prep(Wq, bq, Wk, bk, Wv, bv, gamma)
    zmf = np.ascontiguousarray(zm.reshape(B, C, N).astype(np.float32))
    zcf = np.ascontiguousarray(zc.reshape(B, C, N).astype(np.float32))
    in_maps = [dict(zm=zmf[i], zc=zcf[i], **shared) for i in range(B)]
    res = run_bass_kernel_spmd(nc, in_maps, core_ids=list(range(B)))
    out = np.stack([r["out"] for r in res.results], axis=0)
    return out.reshape(b, c, w, h).astype(np.asarray(zc).dtype)
